# revision 12
# baseline (speedup 1.0000x reference)
"""Trainium2 distributed kernel for AttentionGroupAggregator.

Reference math (N=262144, D=128), sharded row-wise across 8 NeuronCores:
  s   = E @ w_pa + b_pa;   e = exp(s)           (softmax shift skipped: |s|<~0.6)
  S   = sum(e); sum_eE = e^T E                  <- AllReduce #1 ([D+1] floats)
  peer_i = (sum_eE - e_i E_i) / (S - e_i)
  h_i = E_i @ W1^T + peer_i @ W2^T  (+ b_c1 + b_c2, zeros in this problem)
  pi_i = relu(h_i) @ w_vc + b_vc;  sp_i = E_i @ item
  g = exp(sp + pi); Z = sum(g); geE = g^T E     <- AllReduce #2 ([D+1] floats)
  weights = g / Z;  group = geE / Z

Per-core pipeline (rows live on partitions for all per-row scalars):
  phase A: DMA f32 chunks, ScalarE converts to bf16 row-major (erow resident),
  PE-transposes each 128x128 tile into etT (resident), per-tile [s,sp] scores,
  exp on ScalarE, e-weighted row sums accumulated on PE.
  phase B (post AR1): per tile recompute [Y1|Y2] = etT_t^T @ [W1^T|W2^T] on PE;
  ScalarE evacuates Y1 grouped and Y2 with a fused per-row scale (-e*rdn);
  GpSimd adds them; VectorE applies the rank-1 c2sum*rdn term and the fused
  relu-dot (accum_out) producing pi.

Note: gpsimd ucode instructions (affine_select, partition_broadcast, casting
SWDGE DMA) hang under the axon runtime in this container — all avoided.
"""

import os
import sys

for _p in ("/opt/trn_rl_repo",):
    if _p not in sys.path:
        sys.path.insert(0, _p)

import numpy as np

from concourse import bacc, mybir, tile
from concourse.bass_utils import run_bass_kernel_spmd

F32 = mybir.dt.float32
BF16 = mybir.dt.bfloat16
ALU = mybir.AluOpType
ACTF = mybir.ActivationFunctionType

N_CORES = 8
D = 128
P = 128
T_TILES = int(os.environ.get("AGG_T_TILES", "256"))  # 128-row tiles per core
TPC = 16  # tiles per chunk (chunk = one input DMA)
GRP = 4  # tiles per PSUM evacuation group
STAGE = int(os.environ.get("AGG_STAGE", "4"))  # build-bisect gate


class _StageDone(Exception):
    pass


def build_graph(t_tiles=T_TILES):
    r = t_tiles * P
    tpc = min(TPC, t_tiles)
    n_chunks = t_tiles // tpc
    assert tpc * n_chunks == t_tiles and tpc % GRP == 0

    nc = bacc.Bacc(
        "TRN2",
        target_bir_lowering=False,
        debug=False,
        num_devices=N_CORES,
    )

    emb = nc.declare_dram_parameter("emb", [r, D], F32, isOutput=False)
    w1t = nc.declare_dram_parameter("w1t", [D, D], F32, isOutput=False)  # W_c1.T
    w2t = nc.declare_dram_parameter("w2t", [D, D], F32, isOutput=False)  # W_c2.T
    # smalls cols: 0=w_pa, 1=item, 2=b_pa(replicated), 3=b_vc(replicated)
    smalls = nc.declare_dram_parameter("smalls", [D, 4], F32, isOutput=False)
    wvcb = nc.declare_dram_parameter("wvcb", [P, D], F32, isOutput=False)
    identp = nc.declare_dram_parameter("identp", [P, P], F32, isOutput=False)
    weights_o = nc.declare_dram_parameter("weights", [r], F32, isOutput=True)
    group_o = nc.declare_dram_parameter("group", [D], F32, isOutput=True)

    ar1_in = nc.dram_tensor("ar1_in", [D + 1], F32)
    ar1_out = nc.dram_tensor("ar1_out", [D + 1], F32, addr_space="Shared")
    ar2_in = nc.dram_tensor("ar2_in", [D + 1], F32)
    ar2_out = nc.dram_tensor("ar2_out", [D + 1], F32, addr_space="Shared")

    rg = [list(range(N_CORES))]

    # DRAM view: (p, t, d) <-> emb[128t+p, d]
    emb_r = emb.ap().rearrange("(t p) d -> p t d", p=P)
    weights_2d = weights_o.ap().rearrange("(t p) -> t p", p=P)

    try:
        _build_body(nc, t_tiles, tpc, n_chunks, emb_r, weights_2d,
                    ar1_in, ar1_out, ar2_in, ar2_out, rg,
                    smalls, w1t, w2t, wvcb, identp, group_o)
    except _StageDone:
        pass
    nc.compile()
    return nc


def _build_body(nc, t_tiles, tpc, n_chunks, emb_r, weights_2d,
                ar1_in, ar1_out, ar2_in, ar2_out, rg,
                smalls, w1t, w2t, wvcb, identp, group_o):
    r = t_tiles * P
    with tile.TileContext(nc) as tc:
      try:
        with (
            tc.tile_pool(name="big", bufs=1) as big,
            tc.tile_pool(name="consts", bufs=1) as consts,
            tc.tile_pool(name="stats", bufs=1) as stats,
            tc.tile_pool(name="work", bufs=3) as work,
            tc.tile_pool(name="psum_tr", bufs=2, space="PSUM") as psum_tr,
            tc.tile_pool(name="psum_yy", bufs=2, space="PSUM") as psum_yy,
            tc.tile_pool(name="psum_ss", bufs=1, space="PSUM") as psum_ss,
            tc.tile_pool(name="psum_acc", bufs=1, space="PSUM") as psum_acc,
        ):
            # ---------------- constants ----------------
            ident_f = consts.tile([P, P], F32)
            nc.sync.dma_start(ident_f[:], identp[:, :])
            ident = consts.tile([P, P], BF16)
            nc.vector.tensor_copy(ident[:], ident_f[:])

            ones_f = consts.tile([P, 1], F32)
            nc.gpsimd.memset(ones_f[:], 1.0)
            ones_row = consts.tile([1, P], F32)
            nc.gpsimd.memset(ones_row[:], 1.0)
            ones_row_bf = consts.tile([1, P], BF16)
            nc.gpsimd.memset(ones_row_bf[:], 1.0)

            smalls_sb = consts.tile([D, 4], F32)
            nc.sync.dma_start(smalls_sb[:], smalls[:, :])
            wpi_bf = consts.tile([D, 2], BF16)
            nc.vector.tensor_copy(wpi_bf[:], smalls_sb[:, 0:2])

            w1t_f = work.tile([D, D], F32, tag="wstage")
            w2t_f = work.tile([D, D], F32, tag="wstage")
            nc.sync.dma_start(w1t_f[:], w1t[:, :])
            nc.sync.dma_start(w2t_f[:], w2t[:, :])
            w12t = consts.tile([D, 2 * D], BF16)  # [W1^T | W2^T], d on partitions
            nc.vector.tensor_copy(w12t[:, 0:D], w1t_f[:])
            nc.vector.tensor_copy(w12t[:, D : 2 * D], w2t_f[:])

            wvc_f = work.tile([P, D], F32, tag="wstage")
            nc.sync.dma_start(wvc_f[:], wvcb[:, :])
            wvc_b = consts.tile([P, D], BF16)  # w_vc along free axis, all rows
            nc.vector.tensor_copy(wvc_b[:], wvc_f[:])

            # ---------------- big residents (8MB each) ----------------
            erow = big.tile([P, r], BF16)  # row-major: [row%128, (tile, d)]
            etT = big.tile([P, r], BF16)  # transposed: [d, row]

            # ---------------- per-row stats (partition-major [p, tile]) -------
            e_pm = stats.tile([P, t_tiles], F32)
            e_bf = stats.tile([P, t_tiles], BF16)
            sp_pm = stats.tile([P, t_tiles], F32)
            pi_pm = stats.tile([P, t_tiles], F32)
            sc_pm = stats.tile([P, t_tiles], F32)
            g_bf = stats.tile([P, t_tiles], BF16)
            g_f32 = stats.tile([P, t_tiles], F32)
            den_pm = stats.tile([P, t_tiles], F32)
            rdn_pm = stats.tile([P, t_tiles], F32)
            ner_pm = stats.tile([P, t_tiles], F32)

            acc_ps = psum_acc.tile([P, 1], F32)  # sum_eE (phase A) then geE (phase B)

            # ================= PHASE A =================
            for c in range(n_chunks):
                t0 = c * tpc
                sl = slice(t0 * P, (t0 + tpc) * P)
                estg = work.tile([P, tpc, D], F32, tag="estg")
                nc.sync.dma_start(out=estg[:], in_=emb_r[:, t0 : t0 + tpc, :])
                # f32 -> bf16 convert on ScalarE
                erow_3d = erow[:, sl].rearrange("p (t d) -> p t d", d=D)
                nc.scalar.activation(erow_3d, estg[:], ACTF.Copy)

                ss = psum_ss.tile([P, tpc, 2], F32)
                for g in range(tpc // GRP):
                    ptr = psum_tr.tile([P, GRP * P], BF16, tag="ptr")
                    for q in range(GRP):
                        t = t0 + g * GRP + q
                        nc.tensor.transpose(
                            ptr[:, q * P : (q + 1) * P],
                            erow[:, t * P : (t + 1) * P],
                            ident[:],
                        )
                    lo = (t0 + g * GRP) * P
                    # grouped evacuation: PSUM bf16 -> resident etT bf16
                    nc.vector.tensor_copy(etT[:, lo : lo + GRP * P], ptr[:])

                for q in range(tpc):
                    t = t0 + q
                    nc.tensor.matmul(
                        ss[:, q, :], etT[:, t * P : (t + 1) * P], wpi_bf[:]
                    )

                csl = slice(t0, t0 + tpc)
                nc.scalar.activation(
                    e_pm[:, csl], ss[:, :, 0], ACTF.Exp, bias=smalls_sb[:, 2:3]
                )
                nc.scalar.activation(
                    e_bf[:, csl], ss[:, :, 0], ACTF.Exp, bias=smalls_sb[:, 2:3]
                )
                nc.vector.tensor_copy(sp_pm[:, csl], ss[:, :, 1])

                for q in range(tpc):
                    t = t0 + q
                    nc.tensor.matmul(
                        acc_ps[:, 0:1],
                        erow[:, t * P : (t + 1) * P],
                        e_bf[:, t : t + 1],
                        start=(t == 0),
                        stop=(t == t_tiles - 1),
                    )

            # ================= ALLREDUCE 1 =================
            sumeE_sb = stats.tile([P, 1], F32)
            epart = stats.tile([P, 1], F32)
            nc.vector.tensor_reduce(
                epart[:], e_pm[:], mybir.AxisListType.X, ALU.add
            )
            sps_ps = psum_tr.tile([1, 1], F32, tag="ptr")
            nc.tensor.matmul(sps_ps[:], epart[:], ones_f[:])
            if STAGE < 2:
                nc.vector.tensor_copy(sumeE_sb[:], acc_ps[:, 0:1])
                nc.sync.dma_start(group_o[0:D], sumeE_sb[:])
                raise _StageDone()
            s_sb = stats.tile([1, 1], F32)
            nc.vector.tensor_copy(sumeE_sb[:], acc_ps[:, 0:1])
            nc.vector.tensor_copy(s_sb[:], sps_ps[:])
            nc.sync.dma_start(ar1_in[0:D], sumeE_sb[:])
            nc.sync.dma_start(ar1_in[D : D + 1], s_sb[:])
            nc.gpsimd.collective_compute(
                "AllReduce",
                ALU.add,
                replica_groups=rg,
                ins=[ar1_in.ap().opt()],
                outs=[ar1_out.ap().opt()],
            )
            gse_sb = stats.tile([P, 1], F32)  # global sum_eE
            sg_sb = stats.tile([1, 1], F32)  # global S
            nc.sync.dma_start(gse_sb[:], ar1_out[0:D])
            nc.sync.dma_start(sg_sb[:], ar1_out[D : D + 1])

            # ---- post-AR1 small compute ----
            gse_bf = stats.tile([P, 1], BF16)
            nc.vector.tensor_copy(gse_bf[:], gse_sb[:])
            c2_ps = psum_tr.tile([P, 1], F32, tag="ptr")
            nc.tensor.matmul(c2_ps[:], w12t[:, D : 2 * D], gse_bf[:])  # W2 @ sum_eE
            c2_sb = stats.tile([P, 1], BF16)
            nc.vector.tensor_copy(c2_sb[:], c2_ps[:])
            c2t_ps = psum_tr.tile([1, P], BF16, tag="ptr")
            nc.tensor.transpose(c2t_ps[:], c2_sb[:], ident[:])
            c2row = stats.tile([1, P], BF16)
            nc.vector.tensor_copy(c2row[:], c2t_ps[:])
            # broadcast c2 row to all partitions: ones[128,1-col] @ c2row
            c2b_ps = psum_tr.tile([P, P], F32, tag="ptr")
            nc.tensor.matmul(c2b_ps[:], ones_row_bf[:], c2row[:])
            c2sum_b = stats.tile([P, P], BF16)
            nc.scalar.activation(c2sum_b[:], c2b_ps[:], ACTF.Copy)

            # broadcast scalar S to all partitions via f32 matmul
            sb_ps = psum_tr.tile([P, 1], F32, tag="ptr")
            nc.tensor.matmul(sb_ps[:], ones_row[:], sg_sb[:])
            s_b = stats.tile([P, 1], F32)
            nc.vector.tensor_copy(s_b[:], sb_ps[:])

            # den = S - e ; rdn = 1/den ; ner = -e * rdn
            nc.vector.tensor_scalar(
                den_pm[:], e_pm[:], -1.0, s_b[:], ALU.mult, ALU.add
            )
            nc.vector.reciprocal(rdn_pm[:], den_pm[:])
            nc.vector.scalar_tensor_tensor(
                ner_pm[:], e_pm[:], -1.0, rdn_pm[:], ALU.mult, ALU.mult
            )

            if STAGE < 3:
                nc.sync.dma_start(group_o[0:D], s_b[:])
                raise _StageDone()

            # ================= PHASE B =================
            for c in range(n_chunks):
                t0 = c * tpc
                for g in range(tpc // GRP):
                    tg = t0 + g * GRP
                    pyy = psum_yy.tile([P, GRP, 2, D], F32)  # 2 banks
                    for q in range(GRP):
                        t = tg + q
                        nc.tensor.matmul(
                            pyy[:, q],
                            etT[:, t * P : (t + 1) * P],
                            w12t[:],
                        )
                    y1g = work.tile([P, GRP * D], BF16, tag="y1g")
                    t2g = work.tile([P, GRP * D], BF16, tag="t2g")
                    # grouped unscaled Y1 evacuation
                    nc.scalar.activation(y1g[:], pyy[:, :, 0, :], ACTF.Copy)
                    for q in range(GRP):
                        t = tg + q
                        qsl = slice(q * D, (q + 1) * D)
                        # fused scaled evacuation: t2 = -e*rdn * Y2
                        nc.scalar.activation(
                            t2g[:, qsl],
                            pyy[:, q, 1, :],
                            ACTF.Copy,
                            scale=ner_pm[:, t : t + 1],
                        )
                    for q in range(GRP):
                        t = tg + q
                        qsl = slice(q * D, (q + 1) * D)
                        pcomb = work.tile([P, P], BF16, tag="pcomb")
                        hcomb = work.tile([P, P], BF16, tag="hcomb")
                        ztrash = work.tile([P, P], BF16, tag="ztrash")
                        # p = Y1 + t2          (gpsimd)
                        nc.gpsimd.tensor_tensor(
                            pcomb[:], y1g[:, qsl], t2g[:, qsl], ALU.add
                        )
                        # h = c2sum_b * rdn + p      (vector)
                        nc.vector.scalar_tensor_tensor(
                            hcomb[:],
                            c2sum_b[:],
                            rdn_pm[:, t : t + 1],
                            pcomb[:],
                            ALU.mult,
                            ALU.add,
                        )
                        # z = relu(h) * wvc_b ; pi = sum_free(z)   (vector)
                        nc.vector.scalar_tensor_tensor(
                            ztrash[:],
                            hcomb[:],
                            0.0,
                            wvc_b[:],
                            ALU.max,
                            ALU.mult,
                            accum_out=pi_pm[:, t : t + 1],
                        )
                csl = slice(t0, t0 + tpc)
                nc.vector.tensor_tensor(
                    sc_pm[:, csl], pi_pm[:, csl], sp_pm[:, csl], ALU.add
                )
                nc.scalar.activation(
                    g_bf[:, csl], sc_pm[:, csl], ACTF.Exp, bias=smalls_sb[:, 3:4]
                )
                nc.scalar.activation(
                    g_f32[:, csl], sc_pm[:, csl], ACTF.Exp, bias=smalls_sb[:, 3:4]
                )
                for q in range(tpc):
                    t = t0 + q
                    nc.tensor.matmul(
                        acc_ps[:, 0:1],
                        erow[:, t * P : (t + 1) * P],
                        g_bf[:, t : t + 1],
                        start=(t == 0),
                        stop=(t == t_tiles - 1),
                    )

            # ================= ALLREDUCE 2 =================
            geE_sb = stats.tile([P, 1], F32)
            gpart = stats.tile([P, 1], F32)
            nc.vector.tensor_reduce(
                gpart[:], g_f32[:], mybir.AxisListType.X, ALU.add
            )
            zps_ps = psum_tr.tile([1, 1], F32, tag="ptr")
            nc.tensor.matmul(zps_ps[:], gpart[:], ones_f[:])
            if STAGE < 4:
                nc.vector.tensor_copy(geE_sb[:], acc_ps[:, 0:1])
                nc.sync.dma_start(group_o[0:D], geE_sb[:])
                raise _StageDone()
            z_sb = stats.tile([1, 1], F32)
            nc.vector.tensor_copy(geE_sb[:], acc_ps[:, 0:1])
            nc.vector.tensor_copy(z_sb[:], zps_ps[:])
            nc.sync.dma_start(ar2_in[0:D], geE_sb[:])
            nc.sync.dma_start(ar2_in[D : D + 1], z_sb[:])
            nc.gpsimd.collective_compute(
                "AllReduce",
                ALU.add,
                replica_groups=rg,
                ins=[ar2_in.ap().opt()],
                outs=[ar2_out.ap().opt()],
            )
            gge_sb = stats.tile([P, 1], F32)
            zg_sb = stats.tile([1, 1], F32)
            nc.sync.dma_start(gge_sb[:], ar2_out[0:D])
            nc.sync.dma_start(zg_sb[:], ar2_out[D : D + 1])

            # ================= epilogue =================
            rz_sb = stats.tile([1, 1], F32)
            nc.vector.reciprocal(rz_sb[:], zg_sb[:])
            rzb_ps = psum_tr.tile([P, 1], F32, tag="ptr")
            nc.tensor.matmul(rzb_ps[:], ones_row[:], rz_sb[:])
            rz_b = stats.tile([P, 1], F32)
            nc.vector.tensor_copy(rz_b[:], rzb_ps[:])

            grp_sb = stats.tile([P, 1], F32)
            nc.vector.tensor_scalar(grp_sb[:], gge_sb[:], rz_b[:], None, ALU.mult)
            nc.sync.dma_start(group_o[0:D], grp_sb[:])

            wsc = stats.tile([P, t_tiles], F32)
            nc.vector.tensor_scalar(wsc[:], g_f32[:], rz_b[:], None, ALU.mult)
            nblk = (t_tiles + P - 1) // P
            for hblk in range(nblk):
                bw = min(P, t_tiles - hblk * P)
                wt_ps = psum_tr.tile([P, P], F32, tag="ptr")
                nc.tensor.transpose(
                    wt_ps[:bw, :],
                    wsc[:, hblk * P : hblk * P + bw],
                    ident_f[:],
                )
                wt_sb = work.tile([P, P], F32, tag="wtsb")
                nc.vector.tensor_copy(wt_sb[:bw, :], wt_ps[:bw, :])
                nc.sync.dma_start(
                    weights_2d[hblk * P : hblk * P + bw, :], wt_sb[:bw, :]
                )
      except _StageDone:
        pass


_NC_CACHE = {}


def _get_nc(t_tiles):
    if t_tiles not in _NC_CACHE:
        _NC_CACHE[t_tiles] = build_graph(t_tiles)
    return _NC_CACHE[t_tiles]


def _make_in_maps(inputs, t_tiles):
    r = t_tiles * P
    E = np.ascontiguousarray(np.asarray(inputs["member_embeddings"], np.float32))
    smalls = np.zeros((D, 4), dtype=np.float32)
    smalls[:, 0] = np.asarray(inputs["w_pa"], np.float32)
    smalls[:, 1] = np.asarray(inputs["item_embedding"], np.float32)
    smalls[:, 2] = np.float32(inputs["b_pa"])
    smalls[:, 3] = np.float32(inputs["b_vc"])
    w1t = np.ascontiguousarray(np.asarray(inputs["W_c1"], np.float32).T)
    w2t = np.ascontiguousarray(np.asarray(inputs["W_c2"], np.float32).T)
    wvcb = np.ascontiguousarray(
        np.broadcast_to(np.asarray(inputs["w_vc"], np.float32)[None, :], (P, D))
    )
    identp = np.eye(P, dtype=np.float32)
    return [
        {
            "emb": E[c * r : (c + 1) * r],
            "w1t": w1t,
            "w2t": w2t,
            "smalls": smalls,
            "wvcb": wvcb,
            "identp": identp,
        }
        for c in range(N_CORES)
    ]


def kernel(
    member_embeddings,
    item_embedding,
    W_c1,
    b_c1,
    W_c2,
    b_c2,
    w_pa,
    b_pa,
    w_vc,
    b_vc,
):
    E = np.asarray(member_embeddings)
    n, d = E.shape
    assert d == D and n % (N_CORES * P) == 0
    t_tiles = n // (N_CORES * P)
    # b_c1/b_c2 are structurally zero for this problem; fold-check.
    assert float(np.abs(np.asarray(b_c1)).max(initial=0.0)) == 0.0
    assert float(np.abs(np.asarray(b_c2)).max(initial=0.0)) == 0.0

    inputs = dict(
        member_embeddings=member_embeddings,
        item_embedding=item_embedding,
        W_c1=W_c1,
        W_c2=W_c2,
        w_pa=w_pa,
        b_pa=b_pa,
        w_vc=w_vc,
        b_vc=b_vc,
    )
    nc = _get_nc(t_tiles)
    in_maps = _make_in_maps(inputs, t_tiles)
    res = run_bass_kernel_spmd(nc, in_maps, core_ids=list(range(N_CORES)))
    outs = res.results
    weights = np.concatenate([outs[c]["weights"] for c in range(N_CORES)])
    group = outs[0]["group"]
    return group.astype(np.float32), weights.astype(np.float32)


if __name__ == "__main__":
    build_graph(int(os.environ.get("AGG_T_TILES", "16")))
    print("graph built ok")
